# revision 31
# baseline (speedup 1.0000x reference)
"""Trainium2 Bass kernel for nn_CrossAttentionNoGate (v3).

Reference computation (per MSA row s):
    q = split_heads(x_q @ wq); k = split_heads(x_kv @ wk); v = split_heads(x_kv @ wv)
    a = softmax(q k^T/sqrt(D) + (mask-1)*INF + bias)
    out = merge_heads(a @ v) @ wo + bo

Sharding: S=128 rows split 16-per-core across 8 NeuronCores (data parallel);
weights and pair bias replicated.

Per-core design (v3, changes from v2 marked *):
  - x^T ([2C=64*rowparity+ch, token]) is pre-transposed on the HOST and
    DMAed straight to SBUF as bf16; no xbar transposes, no hi/lo merge.
  * projections use row-parity DUPLICATED weights in two K=64 row-groups
    (tile_position (0,0)/(64,0)) so the rp pair runs concurrently on the
    PE instead of a zero-padded serial K=128 matmul.
  - logits are computed transposed ([kv, q]) in bf16 (2 cols/cycle on
    the PE).
  * the pair bias splits between engines to balance them: the g2=0 tiles
    get it PRE-exp on the PE (identity-matmul accumulation as in v2), the
    g2=1 tiles get it POST-exp on DVE as one batched [128,2048] in-place
    multiply by host-precomputed exp(bias^T) (exp(qk+b) = exp(qk)*exp(b)).
    This halves the ~36us/core of identity-matmul PE time and puts the
    other half on DVE at ~16us (bf16 2x throughput, batched).
  - softmax without max-subtraction: ONE exp per [128,1024] psum tile on
    ACT (additive mask as per-partition activation bias), writing bf16.
  - AV: 64-col padded v (col 63 = ones => denominator row), col-tiled at
    out bases {0,64}; kv halves back-to-back per head in one psum bank.
  * denominators gathered to 8 partitions by ONE strided SBUF DMA; the
    bf16->f32 and f32->bf16 companion casts run on GPSIMD; reciprocal on
    DVE; broadcast to [128,1024] f32 psum with a K=8 selector matmul;
    normalize multiply on DVE straight from psum.
  - output projection contracts the padded layout against wo_aug (zero
    rows kill pad/denominator rows): [q-block, 64] natural layout.
  * each row's tail (recip/R/normalize | out-proj) is deferred TWO rows
    (into row s+2's qk-phase | post-AV slot): the denominator-gather DMA
    has ~2.5us of fixed latency, longer than half a row period, so a
    one-row deferral left the tail chain on the critical path.
  * DMA rings: sync ring = weights + expbias + per-row d/out; scalar
    ring = x prefetch only (8 merged xq+xk tiles), so ACT's queue is free
    once the exp stream starts.
"""

import math

import numpy as np

import concourse.bass as bass
import concourse.mybir as mybir
from concourse import bacc as _bacc
import concourse.tile as tile
from concourse import bass_utils

B, S, Q, KV = 1, 128, 256, 256
CQ, CKV = 64, 64
H, D = 8, 32
NCORES = 8
SC = S // NCORES
S2 = SC // 2
INF = 1.0e9
SCALE = 1.0 / math.sqrt(D)

F32 = mybir.dt.float32
F32R = mybir.dt.float32r
BF16 = mybir.dt.bfloat16
EXP = mybir.ActivationFunctionType.Exp

# moving-operand dtype for x (projection inputs): F32R (exact) or BF16 (2x)
X_DT = BF16


def _build(has_bo):
    nc = _bacc.Bacc()

    xT = nc.declare_dram_parameter("xT", [S2, 128, 2, 256], X_DT, isOutput=False)
    biasT = nc.declare_dram_parameter("biasT", [128, 2, 2, 1024], BF16, isOutput=False)
    ident = nc.declare_dram_parameter("ident", [128, 128], BF16, isOutput=False)
    maskcol = nc.declare_dram_parameter("maskcol", [128, SC, 2], F32, isOutput=False)
    esel = nc.declare_dram_parameter("esel", [8, 4, 128], BF16, isOutput=False)
    wq = nc.declare_dram_parameter("wq", [128, 2, 128], BF16, isOutput=False)
    wk = nc.declare_dram_parameter("wk", [128, 2, 128], BF16, isOutput=False)
    wv = nc.declare_dram_parameter("wv", [128, 256], BF16, isOutput=False)
    wot = nc.declare_dram_parameter("wot", [128, 4, CQ], BF16, isOutput=False)
    if has_bo:
        bo1 = nc.declare_dram_parameter("bo1", [1, CQ], F32R, isOutput=False)
    out = nc.declare_dram_parameter("out", [SC, 128, 2, CQ], F32R, isOutput=True)

    from contextlib import ExitStack

    with tile.TileContext(nc) as tc, ExitStack() as ctx:
        def pool(name, bufs, space="SBUF"):
            return ctx.enter_context(tc.tile_pool(name=name, bufs=bufs, space=space))

        singles = pool("singles", 1)
        xin = pool("xin", S2)
        qkp = pool("qk", 2)
        expabp = pool("expab", 6)
        avsbp = pool("avsb", 4)
        otnp = pool("otn", 2)
        drp = pool("dr", 4)
        finp = pool("fin", 3)
        ring = pool("ring", 3, "PSUM")
        avp = pool("avp", 1, "PSUM")

        # ---- constants on the sync HWDGE ring (FIFO, dependency order),
        # then the 1MB expbias tile (needed only by the first post-exp
        # multiply at ~13us).  Per-row d/out DMAs enqueue behind it.
        wq_sb = singles.tile([128, 2, 128], BF16, tag="wq")
        wk_sb = singles.tile([128, 2, 128], BF16, tag="wk")
        wv_sb = singles.tile([128, 256], BF16, tag="wv")
        wo_sb = singles.tile([128, 4, CQ], BF16, tag="wo")
        esel_sb = singles.tile([8, 4, 128], BF16, tag="esel")
        id_sb = singles.tile([128, 128], BF16, tag="id")
        mk_sb = singles.tile([128, SC, 2], F32, tag="mk")
        bias_sb = singles.tile([128, 2, 2, 1024], BF16, tag="biasT")
        nc.sync.dma_start(out=wq_sb[:], in_=wq[:])
        nc.sync.dma_start(out=wk_sb[:], in_=wk[:])
        nc.sync.dma_start(out=wv_sb[:], in_=wv[:])
        nc.sync.dma_start(out=wo_sb[:], in_=wot[:])
        nc.sync.dma_start(out=esel_sb[:], in_=esel[:])
        nc.sync.dma_start(out=id_sb[:], in_=ident[:])
        nc.sync.dma_start(out=mk_sb[:], in_=maskcol[:])
        if has_bo:
            bo_sb = singles.tile([1, CQ], F32R, tag="bo")
            ones_sb = singles.tile([1, 128], F32R, tag="ones")
            nc.sync.dma_start(out=bo_sb[:], in_=bo1[:])
            nc.vector.memset(ones_sb[:], 1.0)
        nc.sync.dma_start(out=bias_sb[:], in_=biasT[:])

        # ---- input prefetch on the second (ACT) HWDGE ring, in parallel
        # with the constants on the sync ring
        x_tiles = []
        for s2 in range(S2):
            x_t = xin.tile([128, 2, 256], X_DT, tag="x")
            nc.scalar.dma_start(out=x_t[:], in_=xT[s2])
            x_tiles.append(x_t)

        # warm the ACT table after the input-DMA dispatch (walrus puts the
        # ~2.7us exp table load before the first ACTIVATE; here it hides
        # under the prefetch drain without delaying the DMA dispatch)
        warm_in = singles.tile([1, 8], F32, tag="warmi")
        warm_out = singles.tile([1, 8], F32, tag="warmo")
        nc.vector.memset(warm_in[:], 0.0)
        nc.scalar.activation(out=warm_out[:], in_=warm_in[:], func=EXP)

        # v tiles: one per row parity, ones column set once, d-cols
        # overwritten each s2 (cols 32..62 hold stale junk that wo_aug's
        # zero rows annihilate)
        v_sb = []
        for vi in range(4):
            vt = singles.tile([128, 2, H, 2 * D], BF16, tag=f"v{vi}")
            nc.vector.memset(vt[:, :, :, D : 2 * D - 1], 0.0)
            nc.vector.memset(vt[:, :, :, 2 * D - 1 : 2 * D], 1.0)
            v_sb.append(vt)

        # Tail of row s, deferred into row s+1 (head-of-line blocking):
        # tailA (recip/R/normalize) issues before row s+1's AV so the
        # latency hides under it; tailB (out-proj) after.
        def make_tails(s, av_sb, d_sb):
            def tailA():
                r_f = drp.tile([H, Q], F32, tag="r")
                r_sr = drp.tile([H, Q], BF16, tag="rr")
                nc.vector.reciprocal_approx_fast(out=r_f[:], in_=d_sb[:])
                nc.vector.tensor_copy(out=r_sr[:], in_=r_f[:])
                R_ps = ring.tile([128, 1024], F32, tag="ps")
                for t4 in range(4):
                    nc.tensor.matmul(
                        R_ps[:, Q * t4 : Q * (t4 + 1)], esel_sb[:, t4, :], r_sr[:]
                    )
                otn = otnp.tile([128, 1024], BF16, tag="otn")
                nc.vector.tensor_mul(otn[:], av_sb[:], R_ps[:])
                return otn

            def tailB(otn):
                fin_ps = ring.tile([128, 2, CQ], F32, tag="ps")
                for qc in range(2):
                    for t4 in range(4):
                        nc.tensor.matmul(
                            fin_ps[:, qc, :],
                            otn[:, Q * t4 + 128 * qc : Q * t4 + 128 * qc + 128],
                            wo_sb[:, t4, :],
                            start=(t4 == 0),
                            stop=(t4 == 3 and not has_bo),
                        )
                    if has_bo:
                        nc.tensor.matmul(
                            fin_ps[:, qc, :], ones_sb[:], bo_sb[:],
                            start=False, stop=True,
                        )
                fin_sb = finp.tile([128, 2, CQ], F32R, tag="fin")
                nc.scalar.copy(out=fin_sb[:], in_=fin_ps[:])
                nc.sync.dma_start(out=out[s], in_=fin_sb[:])

            return tailA, tailB

        from collections import deque

        pending = deque()

        def do_proj(s2):
            # projections: row-parity pairs in two K=64 row-groups run
            # concurrently (distinct PE row bands, distinct PSUM banks)
            x_t = x_tiles[s2]
            xq_t = x_t[:, 0, :]
            xk_t = x_t[:, 1, :]
            qT_ps = ring.tile([128, 2, 2, Q], F32, tag="ps")
            kT_ps = ring.tile([128, 2, 2, KV], F32, tag="ps")
            v_ps = ring.tile([128, 2, 2, 256], F32, tag="ps")
            for rp in range(2):
                sl = slice(64 * rp, 64 * rp + 64)
                for b in range(2):
                    nc.tensor.matmul(
                        qT_ps[:, rp, b, :], wq_sb[sl, b, :], xq_t[sl, :],
                        tile_position=(64 * rp, 0),
                    )
                    nc.tensor.matmul(
                        kT_ps[:, rp, b, :], wk_sb[sl, b, :], xk_t[sl, :],
                        tile_position=(64 * rp, 0),
                    )
            for rp in range(2):
                sl = slice(64 * rp, 64 * rp + 64)
                for ck in range(2):
                    nc.tensor.matmul(
                        v_ps[:, rp, ck, :],
                        xk_t[sl, 128 * ck : 128 * ck + 128],
                        wv_sb[sl, :],
                        tile_position=(64 * rp, 0),
                    )
            # psum -> sbuf: rp0 halves first so the pair's first row never
            # waits on rp1's copies
            qT_sb = qkp.tile([128, 2, 2, Q], BF16, tag="qT")
            kT_sb = qkp.tile([128, 2, 2, 2, 128], BF16, tag="kT")
            for rp in range(2):
                nc.vector.tensor_copy(out=qT_sb[:, rp, :, :], in_=qT_ps[:, rp, :, :])
                nc.vector.tensor_copy(
                    out=kT_sb[:, rp, :, :, :],
                    in_=kT_ps[:, rp, :, :].rearrange("p b (ck r) -> p b ck r", ck=2),
                )
            for rp in range(2):
                nc.vector.tensor_copy(
                    out=v_sb[2 * (s2 % 2) + rp][:, :, :, 0:D],
                    in_=v_ps[:, rp, :, :].rearrange("p ck (h d) -> p ck h d", h=H),
                )
            return qT_sb, kT_sb

        def emit_qk_ck(s, rp, g2, ck, qT_sb, kT_sb, expab):
            # head h -> tile g2=(h%4)//2, bank bk=(h%4)%2, member
            # m=h//4, col 512*bk+256*m, PE row-group 32*(h%4).
            # Same-bank heads {h, h+4} share a row-group (strict serial);
            # cross-bank heads run row-tile concurrent.
            # g2=0: bias pre-added on the PE (identity matmul starts the
            # accumulation group); g2=1: bias multiplied post-exp on DVE
            # (batched over both ck).
            qk = ring.tile([128, 1024], F32, tag="ps")
            if g2 == 0:
                for bk in range(2):
                    nc.tensor.matmul(
                        qk[:, 512 * bk : 512 * bk + 512],
                        id_sb[:],
                        bias_sb[:, ck, 0, 512 * bk : 512 * bk + 512],
                        start=True,
                        stop=False,
                    )
            for m in range(2):
                for bk in range(2):
                    q4 = 2 * g2 + bk
                    nc.tensor.matmul(
                        qk[:, 512 * bk + 256 * m : 512 * bk + 256 * m + 256],
                        kT_sb[32 * q4 : 32 * q4 + 32, rp, m, ck, :],
                        qT_sb[32 * q4 : 32 * q4 + 32, rp, m, :],
                        start=(m == 0 and g2 == 1),
                        stop=(m == 1),
                        tile_position=(32 * q4, 0),
                    )
            nc.scalar.activation(
                out=expab[:, ck, :], in_=qk[:], func=EXP,
                bias=mk_sb[:, s, ck : ck + 1],
            )

        def make_av(s, vrow, expabs):
            # AV: kv halves back-to-back per head; out col-tiled {0,64}.
            # Emitted one row late, in four (g2, m) chunks interleaved
            # between the NEXT row's QK tiles, so the PE never drains while
            # ACT runs this row's exps.
            st = {}

            def group(g2, m):
                if "av" not in st:
                    st["av"] = avp.tile([128, 1024], F32, tag="av", name="av_ps")
                av_ps = st["av"]
                for bk in range(2):
                    h = 4 * m + 2 * g2 + bk
                    t4, u = h // 2, h % 2
                    for ck in range(2):
                        nc.tensor.matmul(
                            av_ps[64 * u : 64 * u + 64, Q * t4 : Q * (t4 + 1)],
                            v_sb[vrow][:, ck, h, :],
                            expabs[g2][
                                :, ck,
                                512 * bk + 256 * m : 512 * bk + 256 * m + 256,
                            ],
                            start=(ck == 0),
                            stop=(ck == 1),
                        )

            def finish():
                # f32 so the denominator gather feeds reciprocal directly
                av_sb = avsbp.tile([128, 1024], F32, tag="avsb")
                nc.vector.tensor_copy(out=av_sb[:], in_=st["av"][:])
                # denominators (rows 63 / 127) -> 8 partitions
                d_sb = drp.tile([H, Q], F32, tag="d")
                for u in range(2):
                    nc.sync.dma_start(
                        out=d_sb[4 * u : 4 * u + 4, :],
                        in_=av_sb[64 * u + 63 : 64 * u + 64, :],
                    )
                pending.append(make_tails(s, av_sb, d_sb))

            return group, finish

        # ---- main loop: rows are software-pipelined with a one-row skew.
        # During row s the PE stream is
        #   qk(1,0) | AV(s-1;1,*) | qk(1,1) | AV(s-1;0,*)+finish |
        #   tailA(s-3) | qk(0,0) | qk(0,1) | tailB(s-3) | [proj at boundary]
        # AV(s-1) completes in the first half of row s so its avcopy (DVE)
        # lands mid-row and row s's own av allocation never stalls; ACT's
        # exp supply never gaps; each qk tile's psum buffer (ring of 3) was
        # freed by an exp ~a full row earlier.
        prev_av = None
        cur_proj = do_proj(0)
        for s2 in range(S2):
            qT_sb, kT_sb = cur_proj
            for rp in range(2):
                s = 2 * s2 + rp
                expabs = {}
                expabs[1] = expabp.tile([128, 2, 1024], BF16, tag="expab",
                                        name="expab1")
                emit_qk_ck(s, rp, 1, 0, qT_sb, kT_sb, expabs[1])
                emit_qk_ck(s, rp, 1, 1, qT_sb, kT_sb, expabs[1])
                nc.vector.tensor_mul(
                    expabs[1][:], expabs[1][:], bias_sb[:, :, 1, :]
                )
                if prev_av is not None:
                    prev_av[0](1, 0)
                    prev_av[0](1, 1)
                tA = tB = potn = None
                if len(pending) == 2:
                    tA, tB = pending.popleft()
                    potn = tA()
                expabs[0] = expabp.tile([128, 2, 1024], BF16, tag="expab",
                                        name="expab0")
                emit_qk_ck(s, rp, 0, 0, qT_sb, kT_sb, expabs[0])
                emit_qk_ck(s, rp, 0, 1, qT_sb, kT_sb, expabs[0])
                if rp == 1 and s2 + 1 < S2:
                    # next pair's projections, one row ahead of use, so
                    # the copies land before row 2*(s2+1) starts
                    cur_proj = do_proj(s2 + 1)
                if prev_av is not None:
                    prev_av[0](0, 0)
                    prev_av[0](0, 1)
                    prev_av[1]()
                if tB is not None:
                    tB(potn)
                prev_av = make_av(s, 2 * (s2 % 2) + rp, expabs)

        # drain: last row's AV, then the remaining tails
        for g2 in (1, 0):
            for m in range(2):
                prev_av[0](g2, m)
        prev_av[1]()
        while pending:
            ptailA, ptailB = pending.popleft()
            ptailB(ptailA())

    nc.finalize()
    return nc


_CACHE = {}


def _get_nc(has_bo):
    if has_bo not in _CACHE:
        _CACHE[has_bo] = _build(has_bo)
    return _CACHE[has_bo]


def _host_prep(input_q, input_kv, mask, bias, wq, wk, wv, wo, bo):
    """Per-core input maps (host-side layout only)."""
    import ml_dtypes

    x_np = np.float32 if X_DT == F32R else ml_dtypes.bfloat16

    # projection weights, row-parity DUPLICATED (two K=64 row-groups)
    wq2 = np.zeros((128, 2, 128), np.float32)
    wk2 = np.zeros((128, 2, 128), np.float32)
    wv2 = np.zeros((128, 256), np.float32)
    for rp in range(2):
        sl = slice(64 * rp, 64 * rp + 64)
        for b in range(2):
            wq2[sl, b, :] = wq.astype(np.float32)[:, 128 * b : 128 * b + 128] * SCALE
            wk2[sl, b, :] = wk.astype(np.float32)[:, 128 * b : 128 * b + 128]
        wv2[sl, :] = wv.astype(np.float32)

    # bias^T bf16: biasT[p, ck, g2, 512*bk + 256*m + q] = bias[h=4m+2g2+bk, q, kv]
    # g2=0 slice stays raw (added pre-exp on the PE); g2=1 slice is
    # exponentiated (multiplied post-exp on DVE).
    bt = bias[0, 0].astype(np.float32)  # [H, Q, KV]
    btT = np.ascontiguousarray(bt.transpose(2, 0, 1))  # [KV, H, Q]
    btT = btT.reshape(2, 128, H, 256)  # [ck, p, h, q]
    perm = np.array([[[0, 4], [1, 5]], [[2, 6], [3, 7]]])  # [g2, bk, m] -> h
    biasT = btT[:, :, perm, :]  # [ck, p, g2, bk, m, q]
    biasT = np.ascontiguousarray(biasT.transpose(1, 0, 2, 3, 4, 5))
    biasT = biasT.reshape(128, 2, 2, 1024).copy()
    biasT[:, :, 1, :] = np.exp(biasT[:, :, 1, :])

    ident_h = np.eye(128, dtype=np.float32)

    # additive mask columns: mk[p, s_local, ck] for kv = 128*ck + p
    mterm = (mask[0, :, 0, 0, :].astype(np.float32) - 1.0) * INF  # [S, KV]
    mterm = mterm.reshape(S, 2, 128).transpose(2, 0, 1)  # [p, s, ck]

    # wo with padded-aug zero rows, partition-major:
    # wot[64u+j, t, c] = wo[(2t+u)*32+j, c], j<32
    wo_t = np.zeros((128, 4, CQ), np.float32)
    for h in range(H):
        t4, u = h // 2, h % 2
        wo_t[64 * u : 64 * u + D, t4, :] = wo[h * D : (h + 1) * D]

    # selector: esel[r, t, 64u+j] = 1 iff r == 4u + t
    esel_h = np.zeros((8, 4, 128), np.float32)
    for t4 in range(4):
        esel_h[t4, t4, 0:64] = 1.0
        esel_h[4 + t4, t4, 64:128] = 1.0

    has_bo = bool(np.any(bo != 0))
    in_maps = []
    for i in range(NCORES):
        sl = slice(SC * i, SC * (i + 1))
        # x^T: [s2, 64*rp + ch, {q|kv}, token]
        xq = input_q[0, sl].astype(np.float32)  # [16, Q, 64]
        xk = input_kv[0, sl].astype(np.float32)
        xqT_h = np.ascontiguousarray(
            xq.reshape(S2, 2, Q, 64).transpose(0, 1, 3, 2).reshape(S2, 128, Q)
        )
        xkT_h = np.ascontiguousarray(
            xk.reshape(S2, 2, KV, 64).transpose(0, 1, 3, 2).reshape(S2, 128, KV)
        )
        xT_h = np.ascontiguousarray(np.stack([xqT_h, xkT_h], axis=2)).astype(x_np)
        m = {
            "xT": xT_h,
            "biasT": biasT.astype(ml_dtypes.bfloat16),
            "ident": ident_h.astype(ml_dtypes.bfloat16),
            "maskcol": np.ascontiguousarray(mterm[:, sl, :]),
            "esel": esel_h.astype(ml_dtypes.bfloat16),
            "wq": wq2.astype(ml_dtypes.bfloat16),
            "wk": wk2.astype(ml_dtypes.bfloat16),
            "wv": wv2.astype(ml_dtypes.bfloat16),
            "wot": wo_t.astype(ml_dtypes.bfloat16),
        }
        if has_bo:
            m["bo1"] = np.ascontiguousarray(bo.astype(np.float32).reshape(1, CQ))
        in_maps.append(m)
    return has_bo, in_maps


def kernel(input_q, input_kv, mask, bias, wq, wk, wv, wo, bo, **_):
    has_bo, in_maps = _host_prep(input_q, input_kv, mask, bias, wq, wk, wv, wo, bo)
    nc = _get_nc(has_bo)
    res = bass_utils.run_bass_kernel_spmd(nc, in_maps, core_ids=list(range(NCORES)))
    outs = []
    for i in range(NCORES):
        o = res.results[i]["out"]  # [SC, 128, 2, CQ]: (s, p, qc, c), q = 128*qc + p
        o = np.asarray(o, np.float32).transpose(0, 2, 1, 3).reshape(SC, Q, CQ)
        outs.append(o)
    full = np.concatenate(outs, axis=0).reshape(B, S, Q, CQ)
    return full.astype(np.float32)


# revision 32
# speedup vs baseline: 1.0002x; 1.0002x over previous
"""Trainium2 Bass kernel for nn_CrossAttentionNoGate (v3).

Reference computation (per MSA row s):
    q = split_heads(x_q @ wq); k = split_heads(x_kv @ wk); v = split_heads(x_kv @ wv)
    a = softmax(q k^T/sqrt(D) + (mask-1)*INF + bias)
    out = merge_heads(a @ v) @ wo + bo

Sharding: S=128 rows split 16-per-core across 8 NeuronCores (data parallel);
weights and pair bias replicated.

Per-core design (v3, changes from v2 marked *):
  - x^T ([2C=64*rowparity+ch, token]) is pre-transposed on the HOST and
    DMAed straight to SBUF as bf16; no xbar transposes, no hi/lo merge.
  * projections use row-parity DUPLICATED weights in two K=64 row-groups
    (tile_position (0,0)/(64,0)) so the rp pair runs concurrently on the
    PE instead of a zero-padded serial K=128 matmul.
  - logits are computed transposed ([kv, q]) in bf16 (2 cols/cycle on
    the PE).
  * the pair bias splits between engines to balance them: the g2=0 tiles
    get it PRE-exp on the PE (identity-matmul accumulation as in v2), the
    g2=1 tiles get it POST-exp on DVE as one batched [128,2048] in-place
    multiply by host-precomputed exp(bias^T) (exp(qk+b) = exp(qk)*exp(b)).
    This halves the ~36us/core of identity-matmul PE time and puts the
    other half on DVE at ~16us (bf16 2x throughput, batched).
  - softmax without max-subtraction: ONE exp per [128,1024] psum tile on
    ACT (additive mask as per-partition activation bias), writing bf16.
  - AV: 64-col padded v (col 63 = ones => denominator row), col-tiled at
    out bases {0,64}; kv halves back-to-back per head in one psum bank.
  * denominators gathered to 8 partitions by ONE strided SBUF DMA; the
    bf16->f32 and f32->bf16 companion casts run on GPSIMD; reciprocal on
    DVE; broadcast to [128,1024] f32 psum with a K=8 selector matmul;
    normalize multiply on DVE straight from psum.
  - output projection contracts the padded layout against wo_aug (zero
    rows kill pad/denominator rows): [q-block, 64] natural layout.
  * each row's tail (recip/R/normalize | out-proj) is deferred TWO rows
    (into row s+2's qk-phase | post-AV slot): the denominator-gather DMA
    has ~2.5us of fixed latency, longer than half a row period, so a
    one-row deferral left the tail chain on the critical path.
  * DMA rings: sync ring = weights + expbias + per-row d/out; scalar
    ring = x prefetch only (8 merged xq+xk tiles), so ACT's queue is free
    once the exp stream starts.
"""

import math

import numpy as np

import concourse.bass as bass
import concourse.mybir as mybir
from concourse import bacc as _bacc
import concourse.tile as tile
from concourse import bass_utils

B, S, Q, KV = 1, 128, 256, 256
CQ, CKV = 64, 64
H, D = 8, 32
NCORES = 8
SC = S // NCORES
S2 = SC // 2
INF = 1.0e9
SCALE = 1.0 / math.sqrt(D)

F32 = mybir.dt.float32
F32R = mybir.dt.float32r
BF16 = mybir.dt.bfloat16
EXP = mybir.ActivationFunctionType.Exp

# moving-operand dtype for x (projection inputs): F32R (exact) or BF16 (2x)
X_DT = BF16


def _build(has_bo):
    nc = _bacc.Bacc()

    xT = nc.declare_dram_parameter("xT", [S2, 128, 2, 256], X_DT, isOutput=False)
    biasT = nc.declare_dram_parameter("biasT", [128, 2, 2, 1024], BF16, isOutput=False)
    ident = nc.declare_dram_parameter("ident", [128, 128], BF16, isOutput=False)
    maskcol = nc.declare_dram_parameter("maskcol", [128, SC, 2], F32, isOutput=False)
    esel = nc.declare_dram_parameter("esel", [8, 4, 128], BF16, isOutput=False)
    wq = nc.declare_dram_parameter("wq", [128, 2, 128], BF16, isOutput=False)
    wk = nc.declare_dram_parameter("wk", [128, 2, 128], BF16, isOutput=False)
    wv = nc.declare_dram_parameter("wv", [128, 256], BF16, isOutput=False)
    wot = nc.declare_dram_parameter("wot", [128, 4, CQ], BF16, isOutput=False)
    if has_bo:
        bo1 = nc.declare_dram_parameter("bo1", [1, CQ], F32R, isOutput=False)
    out = nc.declare_dram_parameter("out", [SC, 128, 2, CQ], F32R, isOutput=True)

    from contextlib import ExitStack

    with tile.TileContext(nc) as tc, ExitStack() as ctx:
        def pool(name, bufs, space="SBUF"):
            return ctx.enter_context(tc.tile_pool(name=name, bufs=bufs, space=space))

        singles = pool("singles", 1)
        xin = pool("xin", S2)
        qkp = pool("qk", 2)
        expabp = pool("expab", 6)
        avsbp = pool("avsb", 4)
        otnp = pool("otn", 2)
        drp = pool("dr", 4)
        finp = pool("fin", 3)
        # PSUM (8 banks): ring = 2 x 2 banks for qk + projection tiles;
        # avp = 2 x 2 banks shared (one tag) by av / R / fin, whose
        # lifetimes are staggered within a row.
        ring = pool("ring", 2, "PSUM")
        avp = pool("avp", 2, "PSUM")

        # ---- constants on the sync HWDGE ring (FIFO, dependency order),
        # then the 1MB expbias tile (needed only by the first post-exp
        # multiply at ~13us).  Per-row d/out DMAs enqueue behind it.
        wq_sb = singles.tile([128, 2, 128], BF16, tag="wq")
        wk_sb = singles.tile([128, 2, 128], BF16, tag="wk")
        wv_sb = singles.tile([128, 256], BF16, tag="wv")
        wo_sb = singles.tile([128, 4, CQ], BF16, tag="wo")
        esel_sb = singles.tile([8, 4, 128], BF16, tag="esel")
        id_sb = singles.tile([128, 128], BF16, tag="id")
        mk_sb = singles.tile([128, SC, 2], F32, tag="mk")
        bias_sb = singles.tile([128, 2, 2, 1024], BF16, tag="biasT")
        nc.sync.dma_start(out=wq_sb[:], in_=wq[:])
        nc.sync.dma_start(out=wk_sb[:], in_=wk[:])
        nc.sync.dma_start(out=wv_sb[:], in_=wv[:])
        nc.sync.dma_start(out=wo_sb[:], in_=wot[:])
        nc.sync.dma_start(out=esel_sb[:], in_=esel[:])
        nc.sync.dma_start(out=id_sb[:], in_=ident[:])
        nc.sync.dma_start(out=mk_sb[:], in_=maskcol[:])
        if has_bo:
            bo_sb = singles.tile([1, CQ], F32R, tag="bo")
            ones_sb = singles.tile([1, 128], F32R, tag="ones")
            nc.sync.dma_start(out=bo_sb[:], in_=bo1[:])
            nc.vector.memset(ones_sb[:], 1.0)
        nc.sync.dma_start(out=bias_sb[:], in_=biasT[:])

        # ---- input prefetch on the second (ACT) HWDGE ring, in parallel
        # with the constants on the sync ring
        x_tiles = []
        for s2 in range(S2):
            x_t = xin.tile([128, 2, 256], X_DT, tag="x")
            nc.scalar.dma_start(out=x_t[:], in_=xT[s2])
            x_tiles.append(x_t)

        # warm the ACT table after the input-DMA dispatch (walrus puts the
        # ~2.7us exp table load before the first ACTIVATE; here it hides
        # under the prefetch drain without delaying the DMA dispatch)
        warm_in = singles.tile([1, 8], F32, tag="warmi")
        warm_out = singles.tile([1, 8], F32, tag="warmo")
        nc.vector.memset(warm_in[:], 0.0)
        nc.scalar.activation(out=warm_out[:], in_=warm_in[:], func=EXP)

        # v tiles: one per row parity, ones column set once, d-cols
        # overwritten each s2 (cols 32..62 hold stale junk that wo_aug's
        # zero rows annihilate)
        v_sb = []
        for vi in range(4):
            vt = singles.tile([128, 2, H, 2 * D], BF16, tag=f"v{vi}")
            nc.vector.memset(vt[:, :, :, D : 2 * D - 1], 0.0)
            nc.vector.memset(vt[:, :, :, 2 * D - 1 : 2 * D], 1.0)
            v_sb.append(vt)

        # Tail of row s, deferred into row s+1 (head-of-line blocking):
        # tailA (recip/R/normalize) issues before row s+1's AV so the
        # latency hides under it; tailB (out-proj) after.
        def make_tails(s, av_sb, d_sb):
            def tailA():
                r_f = drp.tile([H, Q], F32, tag="r")
                r_sr = drp.tile([H, Q], BF16, tag="rr")
                nc.vector.reciprocal_approx_fast(out=r_f[:], in_=d_sb[:])
                nc.vector.tensor_copy(out=r_sr[:], in_=r_f[:])
                R_ps = avp.tile([128, 1024], F32, tag="av")
                for t4 in range(4):
                    nc.tensor.matmul(
                        R_ps[:, Q * t4 : Q * (t4 + 1)], esel_sb[:, t4, :], r_sr[:]
                    )
                otn = otnp.tile([128, 1024], BF16, tag="otn")
                nc.vector.tensor_mul(otn[:], av_sb[:], R_ps[:])
                return otn

            def tailB(otn):
                fin_ps = avp.tile([128, 2, CQ], F32, tag="av")
                for qc in range(2):
                    for t4 in range(4):
                        nc.tensor.matmul(
                            fin_ps[:, qc, :],
                            otn[:, Q * t4 + 128 * qc : Q * t4 + 128 * qc + 128],
                            wo_sb[:, t4, :],
                            start=(t4 == 0),
                            stop=(t4 == 3 and not has_bo),
                        )
                    if has_bo:
                        nc.tensor.matmul(
                            fin_ps[:, qc, :], ones_sb[:], bo_sb[:],
                            start=False, stop=True,
                        )
                fin_sb = finp.tile([128, 2, CQ], F32R, tag="fin")
                nc.scalar.copy(out=fin_sb[:], in_=fin_ps[:])
                nc.sync.dma_start(out=out[s], in_=fin_sb[:])

            return tailA, tailB

        from collections import deque

        pending = deque()

        def do_proj(s2):
            # projections: row-parity pairs in two K=64 row-groups run
            # concurrently (distinct PE row bands, distinct PSUM banks)
            x_t = x_tiles[s2]
            xq_t = x_t[:, 0, :]
            xk_t = x_t[:, 1, :]
            qT_ps = ring.tile([128, 2, 2, Q], F32, tag="ps")
            kT_ps = ring.tile([128, 2, 2, KV], F32, tag="ps")
            v_ps = ring.tile([128, 2, 2, 256], F32, tag="ps")
            for rp in range(2):
                sl = slice(64 * rp, 64 * rp + 64)
                for b in range(2):
                    nc.tensor.matmul(
                        qT_ps[:, rp, b, :], wq_sb[sl, b, :], xq_t[sl, :],
                        tile_position=(64 * rp, 0),
                    )
                    nc.tensor.matmul(
                        kT_ps[:, rp, b, :], wk_sb[sl, b, :], xk_t[sl, :],
                        tile_position=(64 * rp, 0),
                    )
            for rp in range(2):
                sl = slice(64 * rp, 64 * rp + 64)
                for ck in range(2):
                    nc.tensor.matmul(
                        v_ps[:, rp, ck, :],
                        xk_t[sl, 128 * ck : 128 * ck + 128],
                        wv_sb[sl, :],
                        tile_position=(64 * rp, 0),
                    )
            # psum -> sbuf: rp0 halves first so the pair's first row never
            # waits on rp1's copies
            qT_sb = qkp.tile([128, 2, 2, Q], BF16, tag="qT")
            kT_sb = qkp.tile([128, 2, 2, 2, 128], BF16, tag="kT")
            for rp in range(2):
                nc.vector.tensor_copy(out=qT_sb[:, rp, :, :], in_=qT_ps[:, rp, :, :])
                nc.vector.tensor_copy(
                    out=kT_sb[:, rp, :, :, :],
                    in_=kT_ps[:, rp, :, :].rearrange("p b (ck r) -> p b ck r", ck=2),
                )
            for rp in range(2):
                nc.vector.tensor_copy(
                    out=v_sb[2 * (s2 % 2) + rp][:, :, :, 0:D],
                    in_=v_ps[:, rp, :, :].rearrange("p ck (h d) -> p ck h d", h=H),
                )
            return qT_sb, kT_sb

        def emit_qk_ck(s, rp, g2, ck, qT_sb, kT_sb, expab):
            # head h -> tile g2=(h%4)//2, bank bk=(h%4)%2, member
            # m=h//4, col 512*bk+256*m, PE row-group 32*(h%4).
            # Same-bank heads {h, h+4} share a row-group (strict serial);
            # cross-bank heads run row-tile concurrent.
            # g2=0: bias pre-added on the PE (identity matmul starts the
            # accumulation group); g2=1: bias multiplied post-exp on DVE
            # (batched over both ck).
            qk = ring.tile([128, 1024], F32, tag="ps")
            if g2 == 0:
                for bk in range(2):
                    nc.tensor.matmul(
                        qk[:, 512 * bk : 512 * bk + 512],
                        id_sb[:],
                        bias_sb[:, ck, 0, 512 * bk : 512 * bk + 512],
                        start=True,
                        stop=False,
                    )
            for m in range(2):
                for bk in range(2):
                    q4 = 2 * g2 + bk
                    nc.tensor.matmul(
                        qk[:, 512 * bk + 256 * m : 512 * bk + 256 * m + 256],
                        kT_sb[32 * q4 : 32 * q4 + 32, rp, m, ck, :],
                        qT_sb[32 * q4 : 32 * q4 + 32, rp, m, :],
                        start=(m == 0 and g2 == 1),
                        stop=(m == 1),
                        tile_position=(32 * q4, 0),
                    )
            nc.scalar.activation(
                out=expab[:, ck, :], in_=qk[:], func=EXP,
                bias=mk_sb[:, s, ck : ck + 1],
            )

        def make_av(s, vrow, expabs):
            # AV: kv halves back-to-back per head; out col-tiled {0,64}.
            # Emitted one row late, in four (g2, m) chunks interleaved
            # between the NEXT row's QK tiles, so the PE never drains while
            # ACT runs this row's exps.
            st = {}

            def group(g2, m):
                if "av" not in st:
                    st["av"] = avp.tile([128, 1024], F32, tag="av", name="av_ps")
                av_ps = st["av"]
                for bk in range(2):
                    h = 4 * m + 2 * g2 + bk
                    t4, u = h // 2, h % 2
                    for ck in range(2):
                        nc.tensor.matmul(
                            av_ps[64 * u : 64 * u + 64, Q * t4 : Q * (t4 + 1)],
                            v_sb[vrow][:, ck, h, :],
                            expabs[g2][
                                :, ck,
                                512 * bk + 256 * m : 512 * bk + 256 * m + 256,
                            ],
                            start=(ck == 0),
                            stop=(ck == 1),
                        )

            def finish():
                # f32 so the denominator gather feeds reciprocal directly
                av_sb = avsbp.tile([128, 1024], F32, tag="avsb")
                nc.vector.tensor_copy(out=av_sb[:], in_=st["av"][:])
                # denominators (rows 63 / 127) -> 8 partitions
                d_sb = drp.tile([H, Q], F32, tag="d")
                for u in range(2):
                    nc.sync.dma_start(
                        out=d_sb[4 * u : 4 * u + 4, :],
                        in_=av_sb[64 * u + 63 : 64 * u + 64, :],
                    )
                pending.append(make_tails(s, av_sb, d_sb))

            return group, finish

        # ---- main loop: rows are software-pipelined with a one-row skew.
        # During row s the PE stream is
        #   qk(1,0) | AV(s-1;1,*) | qk(1,1) | AV(s-1;0,*)+finish |
        #   tailA(s-3) | qk(0,0) | qk(0,1) | tailB(s-3) | [proj at boundary]
        # AV(s-1) completes in the first half of row s so its avcopy (DVE)
        # lands mid-row and row s's own av allocation never stalls; ACT's
        # exp supply never gaps; each qk tile's psum buffer (ring of 3) was
        # freed by an exp ~a full row earlier.
        prev_av = None
        cur_proj = do_proj(0)
        for s2 in range(S2):
            qT_sb, kT_sb = cur_proj
            for rp in range(2):
                s = 2 * s2 + rp
                expabs = {}
                expabs[1] = expabp.tile([128, 2, 1024], BF16, tag="expab",
                                        name="expab1")
                emit_qk_ck(s, rp, 1, 0, qT_sb, kT_sb, expabs[1])
                emit_qk_ck(s, rp, 1, 1, qT_sb, kT_sb, expabs[1])
                nc.vector.tensor_mul(
                    expabs[1][:], expabs[1][:], bias_sb[:, :, 1, :]
                )
                if prev_av is not None:
                    prev_av[0](1, 0)
                    prev_av[0](1, 1)
                tA = tB = potn = None
                if len(pending) == 2:
                    tA, tB = pending.popleft()
                    potn = tA()
                expabs[0] = expabp.tile([128, 2, 1024], BF16, tag="expab",
                                        name="expab0")
                emit_qk_ck(s, rp, 0, 0, qT_sb, kT_sb, expabs[0])
                emit_qk_ck(s, rp, 0, 1, qT_sb, kT_sb, expabs[0])
                if prev_av is not None:
                    prev_av[0](0, 0)
                    prev_av[0](0, 1)
                    prev_av[1]()
                if tB is not None:
                    tB(potn)
                if rp == 1 and s2 + 1 < S2:
                    cur_proj = do_proj(s2 + 1)
                prev_av = make_av(s, 2 * (s2 % 2) + rp, expabs)

        # drain: last row's AV, then the remaining tails
        for g2 in (1, 0):
            for m in range(2):
                prev_av[0](g2, m)
        prev_av[1]()
        while pending:
            ptailA, ptailB = pending.popleft()
            ptailB(ptailA())

    nc.finalize()
    return nc


_CACHE = {}


def _get_nc(has_bo):
    if has_bo not in _CACHE:
        _CACHE[has_bo] = _build(has_bo)
    return _CACHE[has_bo]


def _host_prep(input_q, input_kv, mask, bias, wq, wk, wv, wo, bo):
    """Per-core input maps (host-side layout only)."""
    import ml_dtypes

    x_np = np.float32 if X_DT == F32R else ml_dtypes.bfloat16

    # projection weights, row-parity DUPLICATED (two K=64 row-groups)
    wq2 = np.zeros((128, 2, 128), np.float32)
    wk2 = np.zeros((128, 2, 128), np.float32)
    wv2 = np.zeros((128, 256), np.float32)
    for rp in range(2):
        sl = slice(64 * rp, 64 * rp + 64)
        for b in range(2):
            wq2[sl, b, :] = wq.astype(np.float32)[:, 128 * b : 128 * b + 128] * SCALE
            wk2[sl, b, :] = wk.astype(np.float32)[:, 128 * b : 128 * b + 128]
        wv2[sl, :] = wv.astype(np.float32)

    # bias^T bf16: biasT[p, ck, g2, 512*bk + 256*m + q] = bias[h=4m+2g2+bk, q, kv]
    # g2=0 slice stays raw (added pre-exp on the PE); g2=1 slice is
    # exponentiated (multiplied post-exp on DVE).
    bt = bias[0, 0].astype(np.float32)  # [H, Q, KV]
    btT = np.ascontiguousarray(bt.transpose(2, 0, 1))  # [KV, H, Q]
    btT = btT.reshape(2, 128, H, 256)  # [ck, p, h, q]
    perm = np.array([[[0, 4], [1, 5]], [[2, 6], [3, 7]]])  # [g2, bk, m] -> h
    biasT = btT[:, :, perm, :]  # [ck, p, g2, bk, m, q]
    biasT = np.ascontiguousarray(biasT.transpose(1, 0, 2, 3, 4, 5))
    biasT = biasT.reshape(128, 2, 2, 1024).copy()
    biasT[:, :, 1, :] = np.exp(biasT[:, :, 1, :])

    ident_h = np.eye(128, dtype=np.float32)

    # additive mask columns: mk[p, s_local, ck] for kv = 128*ck + p
    mterm = (mask[0, :, 0, 0, :].astype(np.float32) - 1.0) * INF  # [S, KV]
    mterm = mterm.reshape(S, 2, 128).transpose(2, 0, 1)  # [p, s, ck]

    # wo with padded-aug zero rows, partition-major:
    # wot[64u+j, t, c] = wo[(2t+u)*32+j, c], j<32
    wo_t = np.zeros((128, 4, CQ), np.float32)
    for h in range(H):
        t4, u = h // 2, h % 2
        wo_t[64 * u : 64 * u + D, t4, :] = wo[h * D : (h + 1) * D]

    # selector: esel[r, t, 64u+j] = 1 iff r == 4u + t
    esel_h = np.zeros((8, 4, 128), np.float32)
    for t4 in range(4):
        esel_h[t4, t4, 0:64] = 1.0
        esel_h[4 + t4, t4, 64:128] = 1.0

    has_bo = bool(np.any(bo != 0))
    in_maps = []
    for i in range(NCORES):
        sl = slice(SC * i, SC * (i + 1))
        # x^T: [s2, 64*rp + ch, {q|kv}, token]
        xq = input_q[0, sl].astype(np.float32)  # [16, Q, 64]
        xk = input_kv[0, sl].astype(np.float32)
        xqT_h = np.ascontiguousarray(
            xq.reshape(S2, 2, Q, 64).transpose(0, 1, 3, 2).reshape(S2, 128, Q)
        )
        xkT_h = np.ascontiguousarray(
            xk.reshape(S2, 2, KV, 64).transpose(0, 1, 3, 2).reshape(S2, 128, KV)
        )
        xT_h = np.ascontiguousarray(np.stack([xqT_h, xkT_h], axis=2)).astype(x_np)
        m = {
            "xT": xT_h,
            "biasT": biasT.astype(ml_dtypes.bfloat16),
            "ident": ident_h.astype(ml_dtypes.bfloat16),
            "maskcol": np.ascontiguousarray(mterm[:, sl, :]),
            "esel": esel_h.astype(ml_dtypes.bfloat16),
            "wq": wq2.astype(ml_dtypes.bfloat16),
            "wk": wk2.astype(ml_dtypes.bfloat16),
            "wv": wv2.astype(ml_dtypes.bfloat16),
            "wot": wo_t.astype(ml_dtypes.bfloat16),
        }
        if has_bo:
            m["bo1"] = np.ascontiguousarray(bo.astype(np.float32).reshape(1, CQ))
        in_maps.append(m)
    return has_bo, in_maps


def kernel(input_q, input_kv, mask, bias, wq, wk, wv, wo, bo, **_):
    has_bo, in_maps = _host_prep(input_q, input_kv, mask, bias, wq, wk, wv, wo, bo)
    nc = _get_nc(has_bo)
    res = bass_utils.run_bass_kernel_spmd(nc, in_maps, core_ids=list(range(NCORES)))
    outs = []
    for i in range(NCORES):
        o = res.results[i]["out"]  # [SC, 128, 2, CQ]: (s, p, qc, c), q = 128*qc + p
        o = np.asarray(o, np.float32).transpose(0, 2, 1, 3).reshape(SC, Q, CQ)
        outs.append(o)
    full = np.concatenate(outs, axis=0).reshape(B, S, Q, CQ)
    return full.astype(np.float32)
